# revision 1
# baseline (speedup 1.0000x reference)
"""BiGCN (graphcl) Trainium2 kernel — 8-core SPMD.

Decomposition (per branch, A = sym-normalized adjacency with self loops):
    h1     = relu(A @ (xv @ W1) + b1)
    pooled = M @ h1 @ W2 + (c_g + 1) * b2        with M = T @ A (dense [B, nv])
    h      = [bu | td];  out = relu(h @ p_w1 + p_b1) @ p_w2 + p_b2

Sharding: destination-node ranges across 8 cores. Layer 1 is computed from
host-staged per-core edge streams (entries = edges + self-loops, sorted by
destination, norm pre-multiplied into the gathered source rows, padded to
128-entry chunks aligned to 128-node tiles; chunk counts maxed over cores so
all 8 cores run one identical program). On device, per chunk:
    x_chunk[128e, 256] @ W1 -> PSUM -> SBUF;  Q = onehot(slot)  (DVE iota==slot)
    h1_tile_psum += Q^T @ xw                   (segment-sum as matmul)
Layer 2 + pooling collapse into G += M_tile^T·h1_tile accumulated in one PSUM
bank; G is the only collective (128KB AllReduce), then the tiny MLP head runs
replicated on every core.
"""
import numpy as np

N_CORES = 8
N = 100000
NV = N + 1
S = 12544                 # nodes per core = 98 * 128
T_TILES = S // 128        # 98
NVP = N_CORES * S
B = 128
IN = 256
HID = 128

BF16 = True              # aggregation-path dtype knob
DEBUG = False


# ----------------------------------------------------------------- host prep
def _build_branch(s_e, d_e, batch):
    deg = np.bincount(d_e, minlength=NV).astype(np.float64) + 1.0
    dinv = 1.0 / np.sqrt(deg)
    es = np.concatenate([s_e, np.arange(NV, dtype=np.int64)])
    ed = np.concatenate([d_e, np.arange(NV, dtype=np.int64)])
    enorm = dinv[es] * dinv[ed]
    order = np.argsort(ed, kind="stable")
    es, ed, enorm = es[order], ed[order], enorm[order]

    M = np.zeros((B, NVP), dtype=np.float64)
    real = ed < N
    np.add.at(M, (batch[ed[real]].astype(np.int64), es[real]), enorm[real])
    virt = ~real
    if virt.any():
        M += np.bincount(es[virt], weights=enorm[virt], minlength=NVP)[None, :]

    # chunking: per (core, 128-node tile), ceil(entries/128), maxed over cores
    tile_of = ed // 128
    counts = np.bincount(tile_of, minlength=N_CORES * T_TILES)
    F = np.maximum(1, -(-counts.reshape(N_CORES, T_TILES) // 128)).max(axis=0)
    C = int(F.sum())
    chunk_base = np.concatenate([[0], np.cumsum(F)])
    tile_starts = np.concatenate([[0], np.cumsum(counts)])

    ent_src = np.zeros((N_CORES, C * 128), dtype=np.int64)
    ent_norm = np.zeros((N_CORES, C * 128), dtype=np.float32)
    ent_slot = np.zeros((N_CORES, C * 128), dtype=np.float32)
    for k in range(N_CORES):
        for t in range(T_TILES):
            gt = k * T_TILES + t
            a, bnd = tile_starts[gt], tile_starts[gt + 1]
            if bnd == a:
                continue
            off = chunk_base[t] * 128
            m = bnd - a
            ent_src[k, off:off + m] = es[a:bnd]
            ent_norm[k, off:off + m] = enorm[a:bnd]
            ent_slot[k, off:off + m] = (ed[a:bnd] - (k * S + t * 128)).astype(np.float32)
    return dict(ent_src=ent_src, ent_norm=ent_norm, ent_slot=ent_slot,
                F=F, C=C, M=M)


def _host_prep(x, emb_w, edge_index, batch):
    xv = np.concatenate([np.asarray(x, np.float32),
                         np.asarray(emb_w, np.float32)], axis=0)
    roots = np.searchsorted(batch, np.arange(B, dtype=batch.dtype)).astype(np.int64)
    ei0 = edge_index[0].astype(np.int64)
    ei1 = edge_index[1].astype(np.int64)
    vs = np.full(B, N, dtype=np.int64)
    br = {
        "td": _build_branch(np.concatenate([ei0, vs]), np.concatenate([ei1, roots]), batch),
        "bu": _build_branch(np.concatenate([ei1, roots]), np.concatenate([ei0, vs]), batch),
    }
    counts_g = np.bincount(batch, minlength=B).astype(np.float64)
    return xv, br, counts_g


# ------------------------------------------------------- walrus wait limiter
def _split_excess_waits(nc, limit=1):
    import concourse.mybir as mybir
    n_added = 0
    for bb in nc.main_func.blocks:
        insts = bb.instructions
        new_list = []
        for inst in insts:
            si = inst.sync_info
            if si is not None and si.on_wait and len(si.on_wait) > limit:
                waits = list(si.on_wait)
                extra, keep = waits[:-limit], waits[-limit:]
                for w in extra:
                    noop = mybir.InstNoOp(name=f"I-wsplit-{nc.next_id()}", ins=[], outs=[])
                    noop.engine = inst.engine
                    noop.sync_info = mybir.SyncInfo(on_wait=[w], on_update=[])
                    nc.register_instruction(noop, overwrite=True)
                    new_list.append(noop)
                    n_added += 1
                inst.sync_info = mybir.SyncInfo(on_wait=keep, on_update=list(si.on_update or []))
            new_list.append(inst)
        insts[:] = new_list
    return n_added


# ------------------------------------------------------------ device program
def _build_program(F_td, F_bu):
    import concourse.bass as bass
    import concourse.mybir as mybir
    import concourse.tile as tile

    f32 = mybir.dt.float32
    dt_s = mybir.dt.bfloat16 if BF16 else f32   # stream / w1 / agg dtype

    nc = bass.Bass(target_bir_lowering=False, trn_type="TRN2", num_swdge_queues=4)

    dram_in = {}
    for bn, C in (("td", int(F_td.sum())), ("bu", int(F_bu.sum()))):
        dram_in[f"xs_{bn}"] = nc.dram_tensor(f"xs_{bn}", [C, IN + 128, 128], dt_s, kind="ExternalInput")
        dram_in[f"mt_{bn}"] = nc.dram_tensor(f"mt_{bn}", [S, 128], dt_s, kind="ExternalInput")
        dram_in[f"w1_{bn}"] = nc.dram_tensor(f"w1_{bn}", [IN, HID], dt_s, kind="ExternalInput")
        dram_in[f"b1b_{bn}"] = nc.dram_tensor(f"b1b_{bn}", [128, HID], f32, kind="ExternalInput")
        dram_in[f"w2_{bn}"] = nc.dram_tensor(f"w2_{bn}", [HID, HID], f32, kind="ExternalInput")
        dram_in[f"pb_{bn}"] = nc.dram_tensor(f"pb_{bn}", [HID, B], f32, kind="ExternalInput")
    dbg = {}
    if DEBUG:
        for nm, shp in (("dbg_g_td", [B, HID]), ("dbg_g_bu", [B, HID]),
                        ("dbg_h1_td", [128, HID]), ("dbg_xws_td", [128, HID]),
                        ("dbg_q_td", [128, 128])):
            dbg[nm] = nc.dram_tensor(nm, shp, f32, kind="ExternalOutput")
    dram_in["pw1"] = nc.dram_tensor("pw1", [2 * HID, 2 * HID], f32, kind="ExternalInput")
    dram_in["pb1"] = nc.dram_tensor("pb1", [128, 2], f32, kind="ExternalInput")
    dram_in["pw2"] = nc.dram_tensor("pw2", [2 * HID, HID], f32, kind="ExternalInput")
    dram_in["pb2"] = nc.dram_tensor("pb2", [128, 1], f32, kind="ExternalInput")
    out_t = nc.dram_tensor("outT", [HID, B], f32, kind="ExternalOutput")

    dma_engines = None  # set per TileContext below

    with tile.TileContext(nc) as tc:
        with (
            tc.tile_pool(name="const", bufs=1) as cpool,
            tc.tile_pool(name="stream", bufs=6) as spool,
            tc.tile_pool(name="work", bufs=6) as wpool,
            tc.tile_pool(name="psA", bufs=3, space="PSUM") as psA,
            tc.tile_pool(name="psH", bufs=3, space="PSUM") as psH,
            tc.tile_pool(name="psG", bufs=2, space="PSUM") as psG,
            tc.tile_pool(name="dram", bufs=1, space="DRAM") as dpool,
        ):
            dma_engines = [nc.sync, nc.scalar, nc.gpsimd]
            dma_rr = [0]

            g_sb = {}
            ar_out = {}
            for bn, F in (("td", F_td), ("bu", F_bu)):
                C = int(F.sum())
                xs = dram_in[f"xs_{bn}"]
                mt_d = dram_in[f"mt_{bn}"]

                w1sb = cpool.tile([128, 2, HID], dt_s, name=f"w1sb_{bn}")
                nc.sync.dma_start(
                    w1sb[:], dram_in[f"w1_{bn}"].rearrange("(kc p) n -> p kc n", p=128))
                b1b4 = cpool.tile([128, 4 * HID], f32, name=f"b1b4_{bn}")
                for _r in range(4):
                    nc.scalar.dma_start(b1b4[:, _r * HID:(_r + 1) * HID],
                                        dram_in[f"b1b_{bn}"][:, :])

                psum_G = psG.tile([HID, B], f32, name=f"psum_G_{bn}", tag="G")
                # chunk c -> 4-chunk cast group; tile t -> 4-tile h1 group
                xws_grp = None
                psum_A = None

                def emit_qmms(lst):
                    for (qq, xg, cc2, ph, tt2, st, sp) in lst:
                        nc.tensor.matmul(ph[:, tt2 * HID:(tt2 + 1) * HID],
                                         qq, xg[:, cc2 * HID:(cc2 + 1) * HID],
                                         start=st, stop=sp)

                def flush_pend(nslices, drain=False):
                    # cast current group, emit PREVIOUS group's Q-matmuls (1-group
                    # software pipeline so PE never waits on the fresh cast)
                    if pend:
                        nc.vector.tensor_copy(xws_grp[:, 0:nslices * HID],
                                              psum_A[:, 0:nslices * HID])
                        emit_qmms(pend_prev)
                        pend_prev[:] = list(pend)
                        pend.clear()
                    if drain:
                        emit_qmms(pend_prev)
                        pend_prev.clear()

                c = 0
                h1_grp = None
                psum_h1 = None
                pend: list = []
                pend_prev: list = []
                pend_m: list = []
                for t in range(T_TILES):
                    tt = t % 4
                    if tt == 0:
                        psum_h1 = psH.tile([128, 4 * HID], f32, name="psum_h1", tag="H")
                        h1_grp = wpool.tile([128, 4 * HID], dt_s, name="h1_grp")
                    ft = int(F[t])
                    for j in range(ft):
                        cc = c % 4
                        if cc == 0:
                            psum_A = psA.tile([128, 4 * HID], f32, name="psum_A", tag="A")
                            xws_grp = wpool.tile([128, 4 * HID], dt_s, name="xws_grp")
                        if c % 4 == 0:
                            nld = min(4, C - c)
                            xt2 = spool.tile([128, 12, 128], dt_s, name="xt2")
                            eng = dma_engines[dma_rr[0] % len(dma_engines)]
                            dma_rr[0] += 1
                            eng.dma_start(
                                xt2[:, 0:nld * 3, :],
                                xs[c:c + nld].rearrange("c2 (kc p) e -> p (c2 kc) e", p=128))
                        sl = (c % 4) * 3
                        for kc in (0, 1):
                            nc.tensor.matmul(psum_A[:, cc * HID:(cc + 1) * HID],
                                             xt2[:, sl + kc, :], w1sb[:, kc, :],
                                             start=(kc == 0), stop=(kc == 1))
                        pend.append((xt2[:, sl + 2, :], xws_grp, cc, psum_h1, tt,
                                      j == 0, j == ft - 1))
                        if cc == 3 or c == C - 1:
                            flush_pend(cc + 1)
                        c += 1
                    if tt == 3 or t == T_TILES - 1:
                        flush_pend(((c - 1) % 4) + 1, drain=True)
                        ns = tt + 1
                        tmp = wpool.tile([128, 4 * HID], f32, name="h1tmp")
                        nc.vector.tensor_tensor(tmp[:, 0:ns * HID], psum_h1[:, 0:ns * HID],
                                                b1b4[:, 0:ns * HID], op=mybir.AluOpType.add)
                        nc.scalar.activation(h1_grp[:, 0:ns * HID], tmp[:, 0:ns * HID],
                                             mybir.ActivationFunctionType.Relu)
                        mt = wpool.tile([128, 4, 128], dt_s, name="mt")
                        t0 = t - tt
                        eng = dma_engines[dma_rr[0] % len(dma_engines)]
                        dma_rr[0] += 1
                        eng.dma_start(mt[:, 0:ns, :],
                                      mt_d[t0 * 128:(t0 + ns) * 128, :].rearrange(
                                          "(jj p) g -> p jj g", p=128))
                        for (mtp, h1p, jj2, st2, sp2) in pend_m:
                            nc.tensor.matmul(psum_G[:], h1p[:, jj2 * HID:(jj2 + 1) * HID],
                                             mtp[:, jj2, :], start=st2, stop=sp2)
                        pend_m = [(mt, h1_grp, jj, t0 + jj == 0, t0 + jj == T_TILES - 1)
                                  for jj in range(ns)]
                        if t == T_TILES - 1:
                            for (mtp, h1p, jj2, st2, sp2) in pend_m:
                                nc.tensor.matmul(psum_G[:], h1p[:, jj2 * HID:(jj2 + 1) * HID],
                                                 mtp[:, jj2, :], start=st2, stop=sp2)
                            pend_m = []
                g = cpool.tile([HID, B], f32, name=f"g_{bn}")
                nc.vector.tensor_copy(g[:], psum_G[:])
                if DEBUG:
                    nc.gpsimd.dma_start(dbg[f"dbg_g_{bn}"][:, :], g[:])
                g_sb[bn] = g
                # per-branch AllReduce: td's runs while bu branch computes
                arin = dpool.tile([HID, B], f32, name=f"arin_{bn}")
                arout = dpool.tile([HID, B], f32, addr_space="Shared", name=f"arout_{bn}")
                nc.gpsimd.dma_start(arin[:], g[:])
                nc.gpsimd.collective_compute(
                    "AllReduce", mybir.AluOpType.add,
                    replica_groups=[list(range(N_CORES))],
                    ins=[arin[:]], outs=[arout[:]],
                )
                ar_out[bn] = arout

            # ---- MLP head (replicated on every core, transposed layout) ----
            pw1 = cpool.tile([128, 2, 2 * HID], f32)
            nc.gpsimd.dma_start(pw1[:], dram_in["pw1"].rearrange("(kc p) n -> p kc n", p=128))
            pb1 = cpool.tile([128, 2], f32)
            nc.gpsimd.dma_start(pb1[:], dram_in["pb1"][:, :])
            pw2 = cpool.tile([128, 2, HID], f32)
            nc.gpsimd.dma_start(pw2[:], dram_in["pw2"].rearrange("(kc p) n -> p kc n", p=128))
            pb2 = cpool.tile([128, 1], f32)
            nc.gpsimd.dma_start(pb2[:], dram_in["pb2"][:, :])

            pool_t = {}
            for i, bn in enumerate(("td", "bu")):
                garr = cpool.tile([HID, B], f32, name=f"garr_{bn}")
                nc.gpsimd.dma_start(garr[:], ar_out[bn][:])
                w2sb = cpool.tile([HID, HID], f32, name=f"w2sb_{bn}")
                nc.gpsimd.dma_start(w2sb[:], dram_in[f"w2_{bn}"][:, :])
                pbsb = cpool.tile([HID, B], f32, name=f"pbsb_{bn}")
                nc.gpsimd.dma_start(pbsb[:], dram_in[f"pb_{bn}"][:, :])

                ps_p = psA.tile([HID, B], f32, name="ps_p", tag="A")
                nc.tensor.matmul(ps_p[:], w2sb[:], garr[:], start=True, stop=True)
                pt = cpool.tile([HID, B], f32, name=f"pool_{bn}")
                nc.vector.tensor_tensor(pt[:], ps_p[:], pbsb[:], op=mybir.AluOpType.add)
                pool_t[bn] = pt                                      # pooled^T [f, g]

            r1 = []
            for hh in range(2):
                ps1 = psA.tile([128, B], f32, name="ps1", tag="A")
                nc.tensor.matmul(ps1[:], pw1[:, 0, hh * 128:(hh + 1) * 128],
                                 pool_t["bu"][:], start=True, stop=False)
                nc.tensor.matmul(ps1[:], pw1[:, 1, hh * 128:(hh + 1) * 128],
                                 pool_t["td"][:], start=False, stop=True)
                r = wpool.tile([128, B], f32, name=f"r1_{hh}")
                nc.scalar.activation(r[:], ps1[:], mybir.ActivationFunctionType.Relu,
                                     bias=pb1[:, hh:hh + 1])
                r1.append(r)
            ps2 = psH.tile([HID, B], f32, name="ps2", tag="H")
            for hh in range(2):
                nc.tensor.matmul(ps2[:], pw2[:, hh, :], r1[hh][:],
                                 start=(hh == 0), stop=(hh == 1))
            ofin = wpool.tile([HID, B], f32, name="ofin")
            nc.vector.tensor_scalar(ofin[:], ps2[:], pb2[:, 0:1], None,
                                    op0=mybir.AluOpType.add)
            nc.gpsimd.dma_start(out_t[:, :], ofin[:])

    _split_excess_waits(nc, limit=1)
    return nc


# ------------------------------------------------------------------- staging
def _stage_core(k, xvp_f32, br, counts_g, inputs, np_dt):
    def cast(a):
        return np.ascontiguousarray(a, dtype=np_dt)

    m = {}
    for bn in ("td", "bu"):
        d = br[bn]
        C = d["C"]
        src = d["ent_src"][k]
        nrm = d["ent_norm"][k]
        xs = xvp_f32[src] * nrm[:, None]                    # [C*128, 256] f32
        xsT = xs.reshape(C, 128, IN).transpose(0, 2, 1)  # [C, 256, 128]
        slot = d["ent_slot"][k].astype(np.int64)
        Q = np.zeros((C * 128, 128), dtype=np.float32)
        Q[np.arange(C * 128), slot] = 1.0
        blob = np.concatenate([xsT, Q.reshape(C, 128, 128)], axis=1)
        m[f"xs_{bn}"] = cast(blob)                       # [C, 384, 128]
        m[f"mt_{bn}"] = cast(d["M"][:, k * S:(k + 1) * S].T)
        w1 = np.asarray(inputs[f"{bn}_w1"], np.float32)
        m[f"w1_{bn}"] = cast(w1)
        m[f"b1b_{bn}"] = np.ascontiguousarray(
            np.broadcast_to(np.asarray(inputs[f"{bn}_b1"], np.float32), (128, HID)))
        m[f"w2_{bn}"] = np.ascontiguousarray(np.asarray(inputs[f"{bn}_w2"], np.float32))
        m[f"pb_{bn}"] = np.ascontiguousarray(
            np.outer(np.asarray(inputs[f"{bn}_b2"], np.float64), counts_g + 1.0),
            dtype=np.float32)
    m["pw1"] = np.ascontiguousarray(np.asarray(inputs["p_w1"], np.float32))
    m["pb1"] = np.ascontiguousarray(
        np.asarray(inputs["p_b1"], np.float32).reshape(2, 128).T)
    m["pw2"] = np.ascontiguousarray(np.asarray(inputs["p_w2"], np.float32))
    m["pb2"] = np.asarray(inputs["p_b2"], np.float32).reshape(128, 1).copy()
    return m


def _enable_ldw_opt():
    import os, stat, tempfile
    from concourse import bass_utils
    if getattr(bass_utils, "_ldw_shim", None):
        return
    real = bass_utils.get_walrus_driver()
    shim = os.path.join(tempfile.gettempdir(), "walrus_ldw_shim.sh")
    with open(shim, "w") as f:
        f.write("#!/bin/sh\nargs=\"\"\nfor a in \"$@\"; do\n"
                "  case \"$a\" in --enable-ldw-opt=false) a=--enable-ldw-opt=true;; esac\n"
                "  args=\"$args $a\"\ndone\nexec %s $args\n" % real)
    os.chmod(shim, stat.S_IRWXU)
    bass_utils.get_walrus_driver = lambda: shim
    bass_utils._ldw_shim = shim


def _run(inputs, trace=False):
    import ml_dtypes
    from concourse import bass_utils

    x = np.asarray(inputs["x"])
    edge_index = np.asarray(inputs["edge_index"])
    batch = np.asarray(inputs["batch"])
    xv, br, counts_g = _host_prep(x, inputs["emb_w"], edge_index, batch)
    xvp = np.zeros((NVP, IN), dtype=np.float32)
    xvp[:NV] = xv

    np_dt = ml_dtypes.bfloat16 if BF16 else np.float32
    in_maps = [_stage_core(k, xvp, br, counts_g, inputs, np_dt)
               for k in range(N_CORES)]
    nc = _build_program(br["td"]["F"], br["bu"]["F"])
    last = None
    for attempt in range(3):
        try:
            res = bass_utils.run_bass_kernel_spmd(
                nc, in_maps, core_ids=list(range(N_CORES)), trace=trace)
            break
        except Exception as e:   # transient NRT device errors recover on retry
            last = e
    else:
        raise last
    out = np.ascontiguousarray(res.results[0]["outT"].T, dtype=np.float32)
    return out, res


def kernel(**inputs) -> np.ndarray:
    out, _ = _run(inputs, trace=False)
    return out

